# revision 24
# baseline (speedup 1.0000x reference)
"""Multi-head causal self-attention (RoPE) on 8 TRN2 NeuronCores.

Strategy (tensor-parallel over heads, per the sharding hint):
  - 16 heads / 8 cores -> 2 heads per core. Each core processes ALL 4
    batches for its 2 heads:
      qkv slice -> RoPE -> causal softmax(q k^T) v -> partial out-proj
    and writes a full-shape partial y (row-parallel w_proj) in bf16.
    The host sums the 8 partials in f32 and adds b_proj.
  - All data is bf16 on the wire and in SBUF; matmul accumulation and
    the RoPE/softmax arithmetic stay in f32 (PSUM / DVE).
  - x is sent pre-transposed (x^T, [C, T] per batch, laid out
    [128, KC, T]) so the contraction dim C lands on SBUF partitions and
    one DMA descriptor fetches a whole 512-token chunk.
  - qkv runs m-group-outer / kc-inner: each of the 6 output blocks
    (q0,q1,k0,k1,v0,v1) accumulates 16 N=512 matmuls into one PSUM bank
    with the weights stationary, then evicts while the next block runs.
    All matmuls are N=512 so the LDWEIGHTS fully hides under the stream.
  - v is produced transposed (v^T, [d, tok]) like q/k and flipped to
    [tok, d] with PE transposes (4 per 512-token chunk per head).
  - Attention in the "S^T" layout (k-tokens on partitions):
      S^T tile   = matmul(lhsT=k^T[:,ktile], rhs=q^T[:,qchunk])
      P^T        = exp(S^T * 1/sqrt(D))      (ACT, no max-subtraction:
                                              |scores| <~ 6 so exp is safe)
      denom      = matmul(lhsT=ones[128,1], rhs=P^T)   (partition sum)
      out^T      = matmul(lhsT=v[ktile,:], rhs=P^T)    accumulated
      attn_out^T = out^T * partition_broadcast(1/denom)
  - The out-projection lags attention by one 512-token chunk (so the
    softmax-rescale tail hides under the next chunk's S matmuls), and
    PSUM pools are allocated once for the whole program (8 banks
    exactly) so there are no pool-boundary sync bubbles.  The diagonal
    128x128 blocks of S/exp/den/out are range-restricted (causal trim).
  - RoPE: the head dim d sits on partitions; rotate_half needs rows
    d <-> d+-64. We permute the d index on the host (within each head's
    128 columns of w_qkv + the cos/sin tables) so that rotation partners
    sit 16 apart inside the same 32-partition quadrant, which a single
    DVE stream_shuffle implements. Scores are invariant to the (shared)
    q/k permutation.
"""

from contextlib import ExitStack

import numpy as np

import concourse.bacc as bacc
import concourse.bass as bass
import concourse.mybir as mybir
import concourse.tile as tile
from concourse.bass import ds, ts

B = 4
T = 2048
C = 2048
H = 16
D = 128
NCORES = 8
HPC = H // NCORES  # heads per core = 2
KC = C // 128  # 16 contraction tiles
TT = T // 128  # 16 token tiles
QCH = 512  # token chunk (qkv stage A and attention stage B)
NQCH = T // QCH
INV_SQRT_D = float(1.0 / np.sqrt(np.float32(D)))

F32 = mybir.dt.float32
BF16 = mybir.dt.bfloat16

# d-permutation: quadrant s holds original d = s*16..s*16+15 (rows 0-15)
# and d+64 partners (rows 16-31); swap = stream_shuffle by +-16.
PERM = np.concatenate(
    [np.concatenate([np.arange(s * 16, s * 16 + 16), 64 + np.arange(s * 16, s * 16 + 16)]) for s in range(4)]
).astype(np.int64)
SWAP_MASK = [(i + 16) % 32 for i in range(32)]


def _mm(nc, out, lhsT, rhs, **kw):
    nc.tensor.matmul(out, lhsT, rhs, **kw)


def build_program():
    nc = bacc.Bacc("TRN2", target_bir_lowering=False, debug=False, num_devices=NCORES)

    xt = nc.dram_tensor("xt", [B, 128, KC, T], BF16, kind="ExternalInput").ap()
    wqkv = nc.dram_tensor("wqkv", [128, KC, 6 * 128], BF16, kind="ExternalInput").ap()
    wproj = nc.dram_tensor("wproj", [HPC, 128, C], BF16, kind="ExternalInput").ap()
    cos_in = nc.dram_tensor("cos_t", [128, T], F32, kind="ExternalInput").ap()
    sin_in = nc.dram_tensor("sin_t", [128, T], F32, kind="ExternalInput").ap()
    masks = nc.dram_tensor("masks", [128, 128], BF16, kind="ExternalInput").ap()
    ones_in = nc.dram_tensor("ones", [128, 1], BF16, kind="ExternalInput").ap()
    ident_in = nc.dram_tensor("ident", [128, 128], BF16, kind="ExternalInput").ap()
    y = nc.dram_tensor("y", [B, TT, 128, C], BF16, kind="ExternalOutput").ap()

    with TileKernel(nc) as tk:
        tk.build(xt, wqkv, wproj, cos_in, sin_in, masks, ones_in, ident_in, y)
    nc.compile()
    return nc


class TileKernel:
    def __init__(self, nc):
        self.nc = nc
        self.stack = ExitStack()

    def __enter__(self):
        self.tc = self.stack.enter_context(tile.TileContext(self.nc))
        return self

    def __exit__(self, *exc):
        return self.stack.__exit__(*exc)

    def build(self, xt, wqkv, wproj, cos_in, sin_in, masks, ones_in, ident_in, y):
        nc, tc = self.nc, self.tc
        ctx = self.stack
        from concourse import library_config
        nc.gpsimd.load_library(library_config.attn)

        consts = ctx.enter_context(tc.tile_pool(name="consts", bufs=1))
        store = ctx.enter_context(tc.tile_pool(name="store", bufs=1))
        self.xtp = ctx.enter_context(tc.tile_pool(name="xtp", bufs=3))
        self.vtp = ctx.enter_context(tc.tile_pool(name="vtp", bufs=2))
        self.ropep = ctx.enter_context(tc.tile_pool(name="ropep", bufs=2))
        self.pp = ctx.enter_context(tc.tile_pool(name="pp", bufs=7))
        self.rp = ctx.enter_context(tc.tile_pool(name="rp", bufs=2))
        self.evp = ctx.enter_context(tc.tile_pool(name="evp", bufs=4))

        # persistent PSUM pools: 3+2+2+1 = 8 banks exactly, never
        # reopened, so there are no pool-boundary sync bubbles.
        self.psW = ctx.enter_context(tc.tile_pool(name="psW", bufs=3, space="PSUM"))
        self.psS = ctx.enter_context(tc.tile_pool(name="psS", bufs=2, space="PSUM"))
        self.psO = ctx.enter_context(tc.tile_pool(name="psO", bufs=2, space="PSUM"))
        self.psD = ctx.enter_context(tc.tile_pool(name="psD", bufs=1, space="PSUM"))

        self.xt_dram = xt
        self.xt_tiles = {}
        self.pending_tr = []

        wqkv_sb = consts.tile([128, KC, 6 * 128], BF16)
        wproj_sb = consts.tile([128, HPC, C], BF16)
        cos_sb = consts.tile([128, T], F32)
        sin_sb = consts.tile([128, T], F32)
        mask_sb = consts.tile([128, 128], BF16)
        ones_col = consts.tile([128, 1], BF16)
        ident_sb = consts.tile([128, 128], BF16)
        # startup order: first weight slices, then the first x chunk in
        # fine slices (the kc=0 matmuls start after ~1MB has landed),
        # then the rest of the weights and the second chunk.
        for kc in range(4):
            nc.sync.dma_start(out=wqkv_sb[:, kc, :], in_=wqkv[:, kc, :])
        self._get_xt(0, 0, split=8)
        for kc in range(4, KC):
            nc.sync.dma_start(out=wqkv_sb[:, kc, :], in_=wqkv[:, kc, :])
        self._get_xt(0, 1, split=4)
        nc.sync.dma_start(out=cos_sb, in_=cos_in)
        nc.sync.dma_start(out=sin_sb, in_=sin_in)
        nc.sync.dma_start(out=ones_col, in_=ones_in)
        nc.sync.dma_start(out=ident_sb, in_=ident_in)

        # ---- per-batch stores (bf16) ----
        q_t = [store.tile([128, T], BF16, name=f"q_t{h}") for h in range(HPC)]
        k_t = [store.tile([128, T], BF16, name=f"k_t{h}") for h in range(HPC)]
        v_sb = [store.tile([128, TT, 128], BF16, name=f"v_sb{h}") for h in range(HPC)]
        ao_t = [store.tile([128, T], BF16, name=f"ao_t{h}") for h in range(HPC)]

        for b in range(B):
            for c in range(NQCH):
                self._stage_a_chunk(b, c, wqkv_sb, cos_sb, sin_sb, ident_sb, q_t, k_t, v_sb)
                if b > 0 and c == 0:
                    # previous batch's last out-proj chunk fills the
                    # batch-boundary bubble
                    self._stage_c(b - 1, NQCH - 1, ao_t, wproj_sb, y)
            self._drain_tr()
            if b == 0:
                # stage-B/C weights load behind batch-0 qkv so the first
                # matmuls aren't queued behind not-yet-needed DMAs
                nc.sync.dma_start(out=mask_sb, in_=masks)
                for h in range(HPC):
                    nc.sync.dma_start(out=wproj_sb[:, h, :], in_=wproj[h])
            if b + 1 < B:
                # prefetch next batch's first chunks behind this batch's
                # attention so the qkv restart isn't DMA-gated
                self._get_xt(b + 1, 0)
                self._get_xt(b + 1, 1)
            for jc in range(NQCH):
                for h in range(HPC):
                    self._stage_b(h, jc, q_t, k_t, v_sb, ao_t, mask_sb, ones_col)
                if jc > 0:
                    # out-proj lags attention by one chunk so the S/o
                    # matmuls hide the softmax-rescale tail
                    self._stage_c(b, jc - 1, ao_t, wproj_sb, y)
        self._stage_c(B - 1, NQCH - 1, ao_t, wproj_sb, y)

    def _get_xt(self, b, c, split=1):
        """One DMA descriptor per 512-token chunk: [128, KC, 512] bf16."""
        key = (b, c)
        if key not in self.xt_tiles:
            nc = self.nc
            t = self.xtp.tile([128, KC, QCH], BF16, tag="xt", name=f"xt_{b}_{c}")
            kcs = KC // split
            for s in range(split):
                nc.sync.dma_start(out=t[:, ds(s * kcs, kcs), :],
                                  in_=self.xt_dram[b, :, ds(s * kcs, kcs), ds(c * QCH, QCH)])
            self.xt_tiles[key] = t
        return self.xt_tiles[key]

    def _drain_tr(self):
        for fn in self.pending_tr:
            fn()
        self.pending_tr = []

    # qkv projection + RoPE for one 512-token chunk of batch b
    def _stage_a_chunk(self, b, c, wqkv_sb, cos_sb, sin_sb, ident_sb, q_t, k_t, v_sb):
        nc = self.nc
        seg = ds(c * QCH, QCH)
        xt_c = self._get_xt(b, c)
        for m in range(6):  # q0 q1 k0 k1 v0 v1
            # deferred v transposes run after the next m-group's matmuls
            # are queued, so the PE isn't stalled on the vt eviction
            self._drain_tr()
            ps = self.psW.tile([128, QCH], F32, tag="w", name="ps_w")
            for kc in range(KC):
                _mm(nc, ps, wqkv_sb[:, kc, ds(m * 128, 128)], xt_c[:, kc, :],
                    start=(kc == 0), stop=(kc == KC - 1))
            if m < 4:
                # RoPE eviction: m -> (q/k, head)
                h = m % 2
                dst = (q_t if m < 2 else k_t)[h]
                sw = self.ropep.tile([128, QCH], F32, tag="sw", name="sw")
                t1 = self.ropep.tile([128, QCH], F32, tag="t1", name="t1")
                nc.vector.stream_shuffle(sw, ps, mask=SWAP_MASK)
                nc.vector.tensor_mul(t1, ps, cos_sb[:, seg])
                nc.vector.tensor_mul(sw, sw, sin_sb[:, seg])
                nc.vector.tensor_add(dst[:, seg], t1, sw)
            else:
                # v^T -> SBUF -> (deferred) PE transpose -> v[tok, d]
                hv = m - 4
                vt = self.vtp.tile([128, QCH], BF16, tag="vt", name="vt")
                nc.scalar.copy(vt, ps)
                self.pending_tr.append(
                    lambda vt=vt, hv=hv, c=c: self._v_transpose(vt, hv, c, ident_sb, v_sb))

    def _v_transpose(self, vt, hv, c, ident_sb, v_sb):
        nc = self.nc
        # bf16 view of a psW bank so the transposes share the big rotation
        pst = self.psW.tile([128, QCH], F32, tag="w", name="ps_w").bitcast(BF16)
        for t in range(4):
            _mm(nc, pst[:, ds(t * 128, 128)], vt[:, ds(t * 128, 128)],
                ident_sb, is_transpose=True,
                start=(t == 0), stop=(t == 3))
        nc.scalar.copy(v_sb[hv][:, ds(c * 4, 4), :], pst[:, ds(0, QCH)])

    # causal attention for head h, q chunk jc (current batch)
    def _stage_b(self, h, jc, q_t, k_t, v_sb, ao_t, mask_sb, ones_col):
        nc = self.nc
        nvalid = (jc + 1) * (QCH // 128)
        # k-tiles in the last 4 (diagonal block) only see q >= r*128 of
        # this chunk; restrict their S/exp/den/out column range.
        def qoff(i):
            return max(0, (i - (nvalid - 4)) * 128)
        ps_d = self.psD.tile([1, QCH], F32, tag="d", name="ps_d")
        ps_o = self.psO.tile([128, QCH], F32, tag="o", name="ps_o")
        # software pipeline: den/out consume ptile a few iterations
        # behind the S-matmul/exp/mask producers so the PE never
        # waits on ACT/DVE.
        LAG = 4
        ptiles = {}
        for i in range(nvalid + LAG):
            if i < nvalid:
                qo = qoff(i)
                qsl = ds(jc * QCH + qo, QCH - qo)
                ps_s = self.psS.tile([128, QCH], F32, tag="s", name="ps_s")
                _mm(nc, ps_s[:, ds(qo, QCH - qo)], k_t[h][:, ds(i * 128, 128)], q_t[h][:, qsl])
                ptile = self.pp.tile([128, QCH], BF16, tag="pt", name="ptile")
                nc.scalar.activation(ptile[:, ds(qo, QCH - qo)], ps_s[:, ds(qo, QCH - qo)],
                                     mybir.ActivationFunctionType.Exp, scale=INV_SQRT_D)
                if i >= nvalid - 4:
                    # lower-tri mask on the leading 128 cols of the
                    # restricted range (the in-tile diagonal)
                    nc.vector.tensor_mul(ptile[:, ds(qo, 128)], ptile[:, ds(qo, 128)], mask_sb)
                ptiles[i] = ptile
            j = i - LAG
            if j >= 0:
                pt = ptiles.pop(j)
                qo = qoff(j)
                _mm(nc, ps_d[:, ds(qo, QCH - qo)], ones_col, pt[:, ds(qo, QCH - qo)],
                    start=(j == 0), stop=(j == nvalid - 1))
                _mm(nc, ps_o[:, ds(qo, QCH - qo)], v_sb[h][:, j, :], pt[:, ds(qo, QCH - qo)],
                    start=(j == 0), stop=(j == nvalid - 1))
        r_sb = self.rp.tile([1, QCH], F32, tag="r", name="r_sb")
        nc.vector.reciprocal_approx_fast(out=r_sb, in_=ps_d)
        rbc = self.rp.tile([128, QCH], F32, tag="rbc", name="rbc")
        nc.gpsimd.partition_broadcast(rbc, r_sb)
        nc.vector.tensor_mul(ao_t[h][:, ds(jc * QCH, QCH)], ps_o, rbc)

    # out-projection partial for batch b, token chunk jc (4 token tiles)
    def _stage_c(self, b, jc, ao_t, wproj_sb, y):
        nc = self.nc
        for tt in range(jc * 4, (jc + 1) * 4):
            yv = self.evp.tile([128, C], BF16, tag="yv", name="yv")
            for nck in range(C // 512):
                ps_y = self.psW.tile([128, 512], F32, tag="w", name="ps_w")
                for h in range(HPC):
                    _mm(nc, ps_y, ao_t[h][:, ds(tt * 128, 128)], wproj_sb[:, h, ds(nck * 512, 512)],
                        start=(h == 0), stop=(h == HPC - 1))
                # alternate eviction engine: ACT alone can't keep pace
                if nck % 2 == 0:
                    nc.scalar.copy(yv[:, ds(nck * 512, 512)], ps_y)
                else:
                    nc.vector.tensor_copy(yv[:, ds(nck * 512, 512)], ps_y)
            nc.sync.dma_start(out=y[b, tt], in_=yv)


def prep_inputs(x, w_qkv, w_proj):
    """Host-side sharding: returns the per-core input maps."""
    import ml_dtypes
    bf = ml_dtypes.bfloat16
    x = np.asarray(x, dtype=np.float32)
    w_qkv = np.asarray(w_qkv, dtype=np.float32)
    w_proj = np.asarray(w_proj, dtype=np.float32)

    # x^T per batch: [B, C, T] -> [B, 128, KC, T] bf16 (partition-major)
    xt = np.ascontiguousarray(
        x.transpose(0, 2, 1).reshape(B, KC, 128, T).transpose(0, 2, 1, 3)
    ).astype(bf)

    # RoPE tables (mirror the fp32 reference computation)
    inv_freq = (1.0 / (10000.0 ** (np.arange(0, D, 2, dtype=np.float32) / D))).astype(np.float32)
    t = np.arange(T, dtype=np.float32)
    freqs = np.einsum("i,j->ij", t, inv_freq).astype(np.float32)  # [T, 64]
    emb = np.concatenate([freqs, freqs], axis=-1)  # [T, 128]
    cos_full = np.cos(emb).astype(np.float32)  # [T, 128]
    sin_full = np.sin(emb).astype(np.float32)
    sgn = np.where(np.arange(D) < D // 2, np.float32(-1.0), np.float32(1.0))
    cos_t = np.ascontiguousarray(cos_full[:, PERM].T)  # [128, T]
    sin_t = np.ascontiguousarray((sin_full * sgn)[:, PERM].T)

    # lower-tri mask for the in-tile diagonal of each 128x128 block
    kp = np.arange(128)[:, None]
    qf = np.arange(128)[None, :]
    masks = (qf >= kp).astype(bf)

    in_maps = []
    for g in range(NCORES):
        heads = [HPC * g + h for h in range(HPC)]
        # wqkv: [C, 768] cols = [q_h0, q_h1, k_h0, k_h1, v_h0, v_h1];
        # q,k d-permuted for RoPE, v unpermuted
        cols = []
        for base in (0, C):  # q block, k block
            for hh in heads:
                cols.append(w_qkv[:, base + hh * 128 + PERM])
        for hh in heads:  # v block
            cols.append(w_qkv[:, 2 * C + hh * 128:2 * C + (hh + 1) * 128])
        wqkv_g = np.ascontiguousarray(
            np.concatenate(cols, axis=1).reshape(KC, 128, 6 * 128).transpose(1, 0, 2)
        ).astype(bf)
        wproj_g = np.ascontiguousarray(
            np.stack([w_proj[hh * 128:(hh + 1) * 128, :] for hh in heads])
        ).astype(bf)
        in_maps.append({
            "xt": xt,
            "wqkv": wqkv_g,
            "wproj": wproj_g,
            "cos_t": cos_t,
            "sin_t": sin_t,
            "masks": masks,
            "ones": np.ones((128, 1), dtype=bf),
            "ident": np.eye(128, dtype=bf),
        })
    return in_maps


_NC_CACHE = {}


def get_program():
    if "nc" not in _NC_CACHE:
        _NC_CACHE["nc"] = build_program()
    return _NC_CACHE["nc"]


def kernel(x, w_qkv, w_proj, b_proj):
    from concourse import bass_utils

    nc = get_program()
    in_maps = prep_inputs(x, w_qkv, w_proj)
    res = bass_utils.run_bass_kernel_spmd(nc, in_maps, core_ids=list(range(NCORES)))
    acc = None
    for r in res.results:
        part = np.asarray(r["y"]).astype(np.float32).reshape(B, T, C)
        acc = part if acc is None else acc + part
    return (acc + np.asarray(b_proj, dtype=np.float32)).astype(np.float32)


# revision 28
# speedup vs baseline: 1.0636x; 1.0636x over previous
"""Multi-head causal self-attention (RoPE) on 8 TRN2 NeuronCores.

Strategy (tensor-parallel over heads, per the sharding hint):
  - 16 heads / 8 cores -> 2 heads per core. Each core processes ALL 4
    batches for its 2 heads:
      qkv slice -> RoPE -> causal softmax(q k^T) v -> partial out-proj
    and writes a full-shape partial y (row-parallel w_proj) in bf16.
    The host sums the 8 partials in f32 and adds b_proj.
  - All data is bf16 on the wire and in SBUF; matmul accumulation and
    the RoPE/softmax arithmetic stay in f32 (PSUM / DVE).
  - x is sent pre-transposed (x^T, [C, T] per batch, laid out
    [128, KC, T]) so the contraction dim C lands on SBUF partitions and
    one DMA descriptor fetches a whole 512-token chunk.
  - qkv runs m-group-outer / kc-inner: each of the 6 output blocks
    (q0,q1,k0,k1,v0,v1) accumulates 16 N=512 matmuls into one PSUM bank
    with the weights stationary, then evicts while the next block runs.
    All matmuls are N=512 so the LDWEIGHTS fully hides under the stream.
  - v is produced transposed (v^T, [d, tok]) like q/k and flipped to
    [tok, d] with PE transposes (4 per 512-token chunk per head).
  - Attention in the "S^T" layout (k-tokens on partitions):
      S^T tile   = matmul(lhsT=k^T[:,ktile], rhs=q^T[:,qchunk])
      P^T        = exp(S^T * 1/sqrt(D))      (ACT, no max-subtraction:
                                              |scores| <~ 6 so exp is safe)
      denom      = matmul(lhsT=ones[128,1], rhs=P^T)   (partition sum)
      out^T      = matmul(lhsT=v[ktile,:], rhs=P^T)    accumulated
      attn_out^T = out^T * partition_broadcast(1/denom)
  - The out-projection lags attention by one 512-token chunk (so the
    softmax-rescale tail hides under the next chunk's S matmuls), and
    PSUM pools are allocated once for the whole program (8 banks
    exactly) so there are no pool-boundary sync bubbles.  The diagonal
    128x128 blocks of S/exp/den/out are range-restricted (causal trim).
  - RoPE: the head dim d sits on partitions; rotate_half needs rows
    d <-> d+-64. We permute the d index on the host (within each head's
    128 columns of w_qkv + the cos/sin tables) so that rotation partners
    sit 16 apart inside the same 32-partition quadrant, which a single
    DVE stream_shuffle implements. Scores are invariant to the (shared)
    q/k permutation.
"""

from contextlib import ExitStack

import numpy as np

import concourse.bacc as bacc
import concourse.bass as bass
import concourse.mybir as mybir
import concourse.tile as tile
from concourse.bass import ds, ts

B = 4
T = 2048
C = 2048
H = 16
D = 128
NCORES = 8
HPC = H // NCORES  # heads per core = 2
KC = C // 128  # 16 contraction tiles
TT = T // 128  # 16 token tiles
QCH = 512  # token chunk (qkv stage A and attention stage B)
NQCH = T // QCH
INV_SQRT_D = float(1.0 / np.sqrt(np.float32(D)))

F32 = mybir.dt.float32
BF16 = mybir.dt.bfloat16

# d-permutation: quadrant s holds original d = s*16..s*16+15 (rows 0-15)
# and d+64 partners (rows 16-31); swap = stream_shuffle by +-16.
PERM = np.concatenate(
    [np.concatenate([np.arange(s * 16, s * 16 + 16), 64 + np.arange(s * 16, s * 16 + 16)]) for s in range(4)]
).astype(np.int64)
SWAP_MASK = [(i + 16) % 32 for i in range(32)]


def _mm(nc, out, lhsT, rhs, **kw):
    nc.tensor.matmul(out, lhsT, rhs, **kw)


def build_program():
    nc = bacc.Bacc("TRN2", target_bir_lowering=False, debug=False, num_devices=NCORES)

    xt = nc.dram_tensor("xt", [B, 128, KC, T], BF16, kind="ExternalInput").ap()
    wqkv = nc.dram_tensor("wqkv", [128, KC, 6 * 128], BF16, kind="ExternalInput").ap()
    wproj = nc.dram_tensor("wproj", [HPC, 128, C], BF16, kind="ExternalInput").ap()
    cos_in = nc.dram_tensor("cos_t", [128, T], F32, kind="ExternalInput").ap()
    sin_in = nc.dram_tensor("sin_t", [128, T], F32, kind="ExternalInput").ap()
    masks = nc.dram_tensor("masks", [128, 128], BF16, kind="ExternalInput").ap()
    ones_in = nc.dram_tensor("ones", [128, 1], BF16, kind="ExternalInput").ap()
    ident_in = nc.dram_tensor("ident", [128, 128], BF16, kind="ExternalInput").ap()
    y = nc.dram_tensor("y", [B, TT, 128, C], BF16, kind="ExternalOutput").ap()

    with TileKernel(nc) as tk:
        tk.build(xt, wqkv, wproj, cos_in, sin_in, masks, ones_in, ident_in, y)
    nc.compile()
    return nc


class TileKernel:
    def __init__(self, nc):
        self.nc = nc
        self.stack = ExitStack()

    def __enter__(self):
        self.tc = self.stack.enter_context(tile.TileContext(self.nc))
        return self

    def __exit__(self, *exc):
        return self.stack.__exit__(*exc)

    def build(self, xt, wqkv, wproj, cos_in, sin_in, masks, ones_in, ident_in, y):
        nc, tc = self.nc, self.tc
        ctx = self.stack
        from concourse import library_config
        nc.gpsimd.load_library(library_config.attn)

        consts = ctx.enter_context(tc.tile_pool(name="consts", bufs=1))
        store = ctx.enter_context(tc.tile_pool(name="store", bufs=1))
        self.xtp = ctx.enter_context(tc.tile_pool(name="xtp", bufs=3))
        self.vtp = ctx.enter_context(tc.tile_pool(name="vtp", bufs=2))
        self.ropep = ctx.enter_context(tc.tile_pool(name="ropep", bufs=2))
        self.pp = ctx.enter_context(tc.tile_pool(name="pp", bufs=7))
        self.dp = ctx.enter_context(tc.tile_pool(name="dp", bufs=2))
        self.rp = ctx.enter_context(tc.tile_pool(name="rp", bufs=2))
        self.evp = ctx.enter_context(tc.tile_pool(name="evp", bufs=4))

        # persistent PSUM pools: 3+2+2+1 = 8 banks exactly, never
        # reopened, so there are no pool-boundary sync bubbles.
        self.psW = ctx.enter_context(tc.tile_pool(name="psW", bufs=3, space="PSUM"))
        self.psS = ctx.enter_context(tc.tile_pool(name="psS", bufs=2, space="PSUM"))
        self.psO = ctx.enter_context(tc.tile_pool(name="psO", bufs=2, space="PSUM"))
        self.psD = ctx.enter_context(tc.tile_pool(name="psD", bufs=1, space="PSUM"))

        self.xt_dram = xt
        self.xt_tiles = {}
        self.pending_tr = []

        wqkv_sb = consts.tile([128, KC, 6 * 128], BF16)
        wproj_sb = consts.tile([128, HPC, C], BF16)
        cos_sb = consts.tile([128, T], F32)
        sin_sb = consts.tile([128, T], F32)
        mask_sb = consts.tile([128, 128], BF16)
        ones_col = consts.tile([128, 1], BF16)
        ident_sb = consts.tile([128, 128], BF16)
        # startup: weights issue from the scalar engine's DGE queue in
        # parallel with the x chunks on the sync queue, halving the
        # descriptor-issue serialization.
        for kc in range(KC):
            nc.scalar.dma_start(out=wqkv_sb[:, kc, :], in_=wqkv[:, kc, :])
        self._get_xt(0, 0, split=8)
        self._get_xt(0, 1, split=4)
        nc.scalar.dma_start(out=cos_sb, in_=cos_in)
        nc.scalar.dma_start(out=sin_sb, in_=sin_in)
        nc.scalar.dma_start(out=ones_col, in_=ones_in)
        nc.scalar.dma_start(out=ident_sb, in_=ident_in)

        # ---- per-batch stores (bf16) ----
        q_t = [store.tile([128, T], BF16, name=f"q_t{h}") for h in range(HPC)]
        k_t = [store.tile([128, T], BF16, name=f"k_t{h}") for h in range(HPC)]
        v_sb = [store.tile([128, TT, 128], BF16, name=f"v_sb{h}") for h in range(HPC)]
        ao_t = [store.tile([128, T], BF16, name=f"ao_t{h}") for h in range(HPC)]

        for b in range(B):
            for c in range(NQCH):
                self._stage_a_chunk(b, c, wqkv_sb, cos_sb, sin_sb, ident_sb, q_t, k_t, v_sb)
                if b > 0 and c == 0:
                    # previous batch's last out-proj chunk fills the
                    # batch-boundary bubble
                    self._stage_c(b - 1, NQCH - 1, ao_t, wproj_sb, y)
            self._drain_tr()
            if b == 0:
                # stage-B/C weights load behind batch-0 qkv so the first
                # matmuls aren't queued behind not-yet-needed DMAs
                nc.scalar.dma_start(out=mask_sb, in_=masks)
                for h in range(HPC):
                    nc.scalar.dma_start(out=wproj_sb[:, h, :], in_=wproj[h])
            if b + 1 < B:
                # prefetch next batch's first chunks behind this batch's
                # attention so the qkv restart isn't DMA-gated
                self._get_xt(b + 1, 0)
                self._get_xt(b + 1, 1)
            for jc in range(NQCH):
                for h in range(HPC):
                    self._stage_b(h, jc, q_t, k_t, v_sb, ao_t, mask_sb, ones_col)
                if jc > 0:
                    # out-proj lags attention by one chunk so the S/o
                    # matmuls hide the softmax-rescale tail
                    self._stage_c(b, jc - 1, ao_t, wproj_sb, y)
        self._stage_c(B - 1, NQCH - 1, ao_t, wproj_sb, y)

    def _get_xt(self, b, c, split=1):
        """One DMA descriptor per 512-token chunk: [128, KC, 512] bf16."""
        key = (b, c)
        if key not in self.xt_tiles:
            nc = self.nc
            t = self.xtp.tile([128, KC, QCH], BF16, tag="xt", name=f"xt_{b}_{c}")
            kcs = KC // split
            for s in range(split):
                nc.sync.dma_start(out=t[:, ds(s * kcs, kcs), :],
                                  in_=self.xt_dram[b, :, ds(s * kcs, kcs), ds(c * QCH, QCH)])
            self.xt_tiles[key] = t
        return self.xt_tiles[key]

    def _drain_tr(self):
        for fn in self.pending_tr:
            fn()
        self.pending_tr = []

    # qkv projection + RoPE for one 512-token chunk of batch b
    def _stage_a_chunk(self, b, c, wqkv_sb, cos_sb, sin_sb, ident_sb, q_t, k_t, v_sb):
        nc = self.nc
        seg = ds(c * QCH, QCH)
        xt_c = self._get_xt(b, c)
        for m in range(6):  # q0 q1 k0 k1 v0 v1
            # deferred v transposes run after the next m-group's matmuls
            # are queued, so the PE isn't stalled on the vt eviction
            self._drain_tr()
            ps = self.psW.tile([128, QCH], F32, tag="w", name="ps_w")
            for kc in range(KC):
                _mm(nc, ps, wqkv_sb[:, kc, ds(m * 128, 128)], xt_c[:, kc, :],
                    start=(kc == 0), stop=(kc == KC - 1))
            if m < 4:
                # RoPE eviction: m -> (q/k, head)
                h = m % 2
                dst = (q_t if m < 2 else k_t)[h]
                sw = self.ropep.tile([128, QCH], F32, tag="sw", name="sw")
                t1 = self.ropep.tile([128, QCH], F32, tag="t1", name="t1")
                nc.vector.stream_shuffle(sw, ps, mask=SWAP_MASK)
                nc.vector.tensor_mul(t1, ps, cos_sb[:, seg])
                nc.vector.tensor_mul(sw, sw, sin_sb[:, seg])
                nc.vector.tensor_add(dst[:, seg], t1, sw)
            else:
                # v^T -> SBUF -> (deferred) PE transpose -> v[tok, d]
                hv = m - 4
                vt = self.vtp.tile([128, QCH], BF16, tag="vt", name="vt")
                nc.scalar.copy(vt, ps)
                self.pending_tr.append(
                    lambda vt=vt, hv=hv, c=c: self._v_transpose(vt, hv, c, ident_sb, v_sb))

    def _v_transpose(self, vt, hv, c, ident_sb, v_sb):
        nc = self.nc
        # bf16 view of a psW bank so the transposes share the big rotation
        pst = self.psW.tile([128, QCH], F32, tag="w", name="ps_w").bitcast(BF16)
        for t in range(4):
            _mm(nc, pst[:, ds(t * 128, 128)], vt[:, ds(t * 128, 128)],
                ident_sb, is_transpose=True,
                start=(t == 0), stop=(t == 3))
        nc.scalar.copy(v_sb[hv][:, ds(c * 4, 4), :], pst[:, ds(0, QCH)])

    # causal attention for head h, q chunk jc (current batch)
    def _stage_b(self, h, jc, q_t, k_t, v_sb, ao_t, mask_sb, ones_col):
        nc = self.nc
        nvalid = (jc + 1) * (QCH // 128)
        # k-tiles in the last 4 (diagonal block) only see q >= r*128 of
        # this chunk; restrict their S/exp/den/out column range.
        def qoff(i):
            return max(0, (i - (nvalid - 4)) * 128)
        ps_d = self.psD.tile([1, QCH], F32, tag="d", name="ps_d")
        ps_o = self.psO.tile([128, QCH], F32, tag="o", name="ps_o")
        # software pipeline: den/out consume ptile a few iterations
        # behind the S-matmul/exp/mask producers so the PE never
        # waits on ACT/DVE.
        LAG = 4
        ptiles = {}
        for i in range(nvalid + LAG):
            if i < nvalid:
                qo = qoff(i)
                qsl = ds(jc * QCH + qo, QCH - qo)
                ps_s = self.psS.tile([128, QCH], F32, tag="s", name="ps_s")
                _mm(nc, ps_s[:, ds(qo, QCH - qo)], k_t[h][:, ds(i * 128, 128)], q_t[h][:, qsl])
                ptile = self.pp.tile([128, QCH], BF16, tag="pt", name="ptile")
                nc.scalar.activation(ptile[:, ds(qo, QCH - qo)], ps_s[:, ds(qo, QCH - qo)],
                                     mybir.ActivationFunctionType.Exp, scale=INV_SQRT_D)
                if i >= nvalid - 4:
                    # lower-tri mask on the leading 128 cols of the
                    # restricted range (the in-tile diagonal)
                    nc.vector.tensor_mul(ptile[:, ds(qo, 128)], ptile[:, ds(qo, 128)], mask_sb)
                ptiles[i] = ptile
            j = i - LAG
            if j >= 0:
                pt = ptiles.pop(j)
                qo = qoff(j)
                if j < nvalid - 4:
                    # off-diagonal: halve the denominator matmuls by
                    # summing ptile pairs on the DVE first
                    if j % 2 == 0:
                        pair0 = pt
                    else:
                        pq = self.dp.tile([128, QCH], BF16, tag="pq", name="pq")
                        nc.vector.tensor_add(pq, pair0, pt)
                        _mm(nc, ps_d, ones_col, pq, start=(j == 1), stop=False)
                else:
                    _mm(nc, ps_d[:, ds(qo, QCH - qo)], ones_col, pt[:, ds(qo, QCH - qo)],
                        start=(j == 0), stop=(j == nvalid - 1))
                _mm(nc, ps_o[:, ds(qo, QCH - qo)], v_sb[h][:, j, :], pt[:, ds(qo, QCH - qo)],
                    start=(j == 0), stop=(j == nvalid - 1))
        r_sb = self.rp.tile([1, QCH], F32, tag="r", name="r_sb")
        nc.vector.reciprocal_approx_fast(out=r_sb, in_=ps_d)
        rbc = self.rp.tile([128, QCH], F32, tag="rbc", name="rbc")
        nc.gpsimd.partition_broadcast(rbc, r_sb)
        nc.vector.tensor_mul(ao_t[h][:, ds(jc * QCH, QCH)], ps_o, rbc)

    # out-projection partial for batch b, token chunk jc (4 token tiles)
    def _stage_c(self, b, jc, ao_t, wproj_sb, y):
        nc = self.nc
        for tt in range(jc * 4, (jc + 1) * 4):
            yv = self.evp.tile([128, C], BF16, tag="yv", name="yv")
            for nck in range(C // 512):
                ps_y = self.psW.tile([128, 512], F32, tag="w", name="ps_w")
                for h in range(HPC):
                    _mm(nc, ps_y, ao_t[h][:, ds(tt * 128, 128)], wproj_sb[:, h, ds(nck * 512, 512)],
                        start=(h == 0), stop=(h == HPC - 1))
                # alternate eviction engine: ACT alone can't keep pace
                if nck % 2 == 0:
                    nc.scalar.copy(yv[:, ds(nck * 512, 512)], ps_y)
                else:
                    nc.vector.tensor_copy(yv[:, ds(nck * 512, 512)], ps_y)
            nc.sync.dma_start(out=y[b, tt], in_=yv)


def prep_inputs(x, w_qkv, w_proj):
    """Host-side sharding: returns the per-core input maps."""
    import ml_dtypes
    bf = ml_dtypes.bfloat16
    x = np.asarray(x, dtype=np.float32)
    w_qkv = np.asarray(w_qkv, dtype=np.float32)
    w_proj = np.asarray(w_proj, dtype=np.float32)

    # x^T per batch: [B, C, T] -> [B, 128, KC, T] bf16 (partition-major)
    xt = np.ascontiguousarray(
        x.transpose(0, 2, 1).reshape(B, KC, 128, T).transpose(0, 2, 1, 3)
    ).astype(bf)

    # RoPE tables (mirror the fp32 reference computation)
    inv_freq = (1.0 / (10000.0 ** (np.arange(0, D, 2, dtype=np.float32) / D))).astype(np.float32)
    t = np.arange(T, dtype=np.float32)
    freqs = np.einsum("i,j->ij", t, inv_freq).astype(np.float32)  # [T, 64]
    emb = np.concatenate([freqs, freqs], axis=-1)  # [T, 128]
    cos_full = np.cos(emb).astype(np.float32)  # [T, 128]
    sin_full = np.sin(emb).astype(np.float32)
    sgn = np.where(np.arange(D) < D // 2, np.float32(-1.0), np.float32(1.0))
    cos_t = np.ascontiguousarray(cos_full[:, PERM].T)  # [128, T]
    sin_t = np.ascontiguousarray((sin_full * sgn)[:, PERM].T)

    # lower-tri mask for the in-tile diagonal of each 128x128 block
    kp = np.arange(128)[:, None]
    qf = np.arange(128)[None, :]
    masks = (qf >= kp).astype(bf)

    in_maps = []
    for g in range(NCORES):
        heads = [HPC * g + h for h in range(HPC)]
        # wqkv: [C, 768] cols = [q_h0, q_h1, k_h0, k_h1, v_h0, v_h1];
        # q,k d-permuted for RoPE, v unpermuted
        cols = []
        for base in (0, C):  # q block, k block
            for hh in heads:
                cols.append(w_qkv[:, base + hh * 128 + PERM])
        for hh in heads:  # v block
            cols.append(w_qkv[:, 2 * C + hh * 128:2 * C + (hh + 1) * 128])
        wqkv_g = np.ascontiguousarray(
            np.concatenate(cols, axis=1).reshape(KC, 128, 6 * 128).transpose(1, 0, 2)
        ).astype(bf)
        wproj_g = np.ascontiguousarray(
            np.stack([w_proj[hh * 128:(hh + 1) * 128, :] for hh in heads])
        ).astype(bf)
        in_maps.append({
            "xt": xt,
            "wqkv": wqkv_g,
            "wproj": wproj_g,
            "cos_t": cos_t,
            "sin_t": sin_t,
            "masks": masks,
            "ones": np.ones((128, 1), dtype=bf),
            "ident": np.eye(128, dtype=bf),
        })
    return in_maps


_NC_CACHE = {}


def get_program():
    if "nc" not in _NC_CACHE:
        _NC_CACHE["nc"] = build_program()
    return _NC_CACHE["nc"]


def kernel(x, w_qkv, w_proj, b_proj):
    from concourse import bass_utils

    nc = get_program()
    in_maps = prep_inputs(x, w_qkv, w_proj)
    res = bass_utils.run_bass_kernel_spmd(nc, in_maps, core_ids=list(range(NCORES)))
    acc = None
    for r in res.results:
        part = np.asarray(r["y"]).astype(np.float32).reshape(B, T, C)
        acc = part if acc is None else acc + part
    return (acc + np.asarray(b_proj, dtype=np.float32)).astype(np.float32)
